# revision 1
# baseline (speedup 1.0000x reference)
"""Deformable KPConv layer, data-parallel over query points on 8 NeuronCores.

Strategy (per sharding hint): shard N across the 8 cores; each core keeps a
replicated copy of the support_points/x tables and all kernel weights, gathers
its neighbors locally, and computes its slice of the output. No collectives
needed; the host concatenates the 8 output shards.
"""
import numpy as np
from functools import partial

import jax
import jax.numpy as jnp

POINT_INFLUENCE = 1.0
N_KP = 15
DIM = 3
N_CORES = 8


def _kpconv_core(q, s_tab, neigh, x_tab, kpts, W, extent):
    nb = s_tab[neigh] - q[:, None, :]                      # [n,M,3]
    if kpts.ndim == 2:
        diff = nb[:, :, None, :] - kpts[None, None, :, :]  # [n,M,K,3]
    else:
        diff = nb[:, :, None, :] - kpts[:, None, :, :]
    sqd = jnp.sum(diff * diff, axis=-1)                    # [n,M,K]
    aw = jnp.maximum(1.0 - jnp.sqrt(sqd) / extent, 0.0)
    aw = jnp.swapaxes(aw, 1, 2)                            # [n,K,M]
    nf = x_tab[neigh]                                      # [n,M,Din]
    wf = jnp.einsum('nkm,nmd->nkd', aw, nf)
    return jnp.einsum('nkd,kde->ne', wf, W)


@partial(jax.pmap, in_axes=(0, None, 0, None, None, None, None, None))
def _shard_fn(q, s_tab, neigh, x_tab, kp, off_w, off_b, w):
    off_feat = _kpconv_core(q, s_tab, neigh, x_tab, kp, off_w, POINT_INFLUENCE) + off_b
    offsets = off_feat.reshape(-1, N_KP, DIM) * POINT_INFLUENCE
    deformed = kp[None, :, :] + offsets                    # [n,K,3]
    return _kpconv_core(q, s_tab, neigh, x_tab, deformed, w, POINT_INFLUENCE)


def kernel(query_points, support_points, neighbors, x, K_points,
           offset_weights, offset_bias, weight):
    N = query_points.shape[0]
    S = N // N_CORES
    assert S * N_CORES == N

    q = np.ascontiguousarray(np.asarray(query_points, np.float32).reshape(N_CORES, S, DIM))
    neigh = np.ascontiguousarray(np.asarray(neighbors).astype(np.int32).reshape(N_CORES, S, -1))
    out = _shard_fn(
        q,
        np.asarray(support_points, np.float32),
        neigh,
        np.asarray(x, np.float32),
        np.asarray(K_points, np.float32),
        np.asarray(offset_weights, np.float32),
        np.asarray(offset_bias, np.float32),
        np.asarray(weight, np.float32),
    )
    return np.asarray(out).reshape(N, -1)
